# revision 17
# baseline (speedup 1.0000x reference)
"""Trainium2 Bass kernel for nn_Attention (dense transformer attention block).

Full causal attention: QKV projection + RoPE + softmax(QK^T/sqrt(d) + mask)V + WO,
bsz=1, seqlen=2048, dim=4096, 32 heads x head_dim 128, fp32 I/O.

Sharding: tensor-parallel across heads on 8 NeuronCores. Core c owns heads
4c..4c+3 (wq/wk/wv output columns, attention) and wo output columns
512c..512c+512 (after an AllGather of the per-core attn^T shard along the
head axis). Host concatenates the 8 output column shards.

Schedule (causal fast path): the PE is the bottleneck (board power throttle
caps it at ~1.95GHz), so the emission order is built to keep it gap-free:
  - Q kept resident in SBUF (no DRAM spill round-trip).
  - V-projection is emitted as small filler chunks between the score and
    PV matmuls of each attention head, covering the exp-chain latency.
  - Causal diagonal-band tiles compute only the unmasked column suffix;
    masking is a multiplicative 0/1 triangle on the diagonal block.
  - wo(1)/wo(2) are held back until after the last AllGather is issued so
    the collective hides under them; the tail is a single wo pass.
"""

import contextlib

import ml_dtypes
import numpy as np

import concourse.bacc as bacc
import concourse.mybir as mybir
import concourse.tile as tile
from concourse.bass_utils import run_bass_kernel_spmd

# Problem constants (hardcoded per contract)
N_CORES = 8
S = 2048              # sequence length
D = 4096              # model dim
HD = 128              # head dim
NH_LOC = 4            # heads per core
DSH = 512             # per-core shard width (NH_LOC * HD)
KT = D // 128         # 32 contraction tiles over model dim
QTILES = S // 128     # 16 token tiles
QRANGES = S // 512    # 4 query ranges of 512
SCALE = float(1.0 / np.sqrt(HD))

F32 = mybir.dt.float32
F32R = mybir.dt.float32r
BF16 = mybir.dt.bfloat16

_PROGRAMS = {}


def _build_causal():
    nc = bacc.Bacc("TRN2", target_bir_lowering=False, debug=False,
                   num_devices=N_CORES)

    # ---- external inputs (per core) ----
    xT_d = nc.dram_tensor("xT", [D, S], BF16, kind="ExternalInput")
    wq_d = nc.dram_tensor("wq", [NH_LOC, 128, KT, HD], BF16, kind="ExternalInput")
    wk_d = nc.dram_tensor("wk", [NH_LOC, 128, KT, HD], BF16, kind="ExternalInput")
    wv_d = nc.dram_tensor("wv", [128, KT, DSH], BF16, kind="ExternalInput")
    wo_d = nc.dram_tensor("wo", [128, KT, DSH], BF16, kind="ExternalInput")
    fr_d = nc.dram_tensor("fr128", [128, S], BF16, kind="ExternalInput")
    fis_d = nc.dram_tensor("fis128", [128, S], BF16, kind="ExternalInput")
    ones_d = nc.dram_tensor("onesmat", [128, 128], BF16, kind="ExternalInput")
    tri_d = nc.dram_tensor("tri01", [128, 128], BF16, kind="ExternalInput")
    out_d = nc.dram_tensor("out", [S, DSH], F32, kind="ExternalOutput")

    with tile.TileContext(nc) as tc, contextlib.ExitStack() as es:
        # ---- persistent pools (left side) ----
        cns = es.enter_context(tc.tile_pool(name="consts", bufs=1, side="left"))
        dram = es.enter_context(tc.tile_pool(name="dram", bufs=1, space="DRAM"))
        akv = es.enter_context(tc.tile_pool(name="akv", bufs=1, side="left"))
        aptA = es.enter_context(tc.tile_pool(name="aptA", bufs=8, side="left"))
        awk = es.enter_context(tc.tile_pool(name="awk", bufs=2, side="left"))
        ps = es.enter_context(tc.tile_pool(name="ps", bufs=1, space="PSUM"))

        agi = [dram.tile([DSH, 512], BF16, name=f"agi{r}") for r in range(4)]
        ago = [dram.tile([D, 512], BF16, addr_space="Shared", name=f"ago{r}")
               for r in range(4)]

        ones_sb = cns.tile([128, 128], BF16, tag="ones")
        tri_sb = cns.tile([128, 128], BF16, tag="tri")

        kts = [akv.tile([128, S], BF16, tag=f"kth{h}", name=f"kth{h}")
               for h in range(NH_LOC)]
        qts = [akv.tile([128, S], BF16, tag=f"qth{h}", name=f"qth{h}")
               for h in range(NH_LOC)]

        # ---- transient pool: x^T fully resident (left, 128KB/part) ----
        xtp_cm = tc.tile_pool(name="xtp", bufs=1, side="left")
        xtp = xtp_cm.__enter__()
        xt_sb = xtp.tile([128, KT, S], BF16, tag="xt")

        # ================= Section 1: Q/K projections + RoPE =================
        with (
            tc.tile_pool(name="qkc", bufs=1, side="left") as qkc,
            tc.tile_pool(name="qkw", bufs=4, side="left") as qkw,
            tc.tile_pool(name="qkd", bufs=2, side="left") as qkd,
        ):
            fr_sb = qkc.tile([128, S], BF16, tag="fr")
            fis_sb = qkc.tile([128, S], BF16, tag="fis")

            # first kt tile split into 4 column quarters on 4 queues so the
            # very first matmul's moving operand lands as early as possible
            for qi, eng in enumerate((nc.sync, nc.scalar, nc.gpsimd, nc.sync)):
                eng.dma_start(
                    xt_sb[:, 0, qi * 512:(qi + 1) * 512],
                    xT_d[0:128, qi * 512:(qi + 1) * 512],
                )
            xt_chunks = [(1, 2), (2, 4), (4, 7), (7, 11), (11, 16),
                         (16, 22), (22, 28), (28, 32)]
            for ch, (k0, k1) in enumerate(xt_chunks):
                nc.sync.dma_start(
                    xt_sb[:, k0:k1, :],
                    xT_d[k0 * 128:k1 * 128, :]
                    .rearrange("(kt p) s -> p kt s", p=128),
                )
                if ch == 0:
                    nc.gpsimd.dma_start(fr_sb[:], fr_d[:, :])
                    nc.gpsimd.dma_start(fis_sb[:], fis_d[:, :])

            # oi pairs in kt-major order: halves the x-consumption rate so
            # the first pass never outruns the x DMA stream.
            ps_tags = [[f"a{j}" for j in range(4)],
                       ["b", "b", "c", "c"]]
            consts_loaded = [False]
            for pr in range(4):
                ois = (2 * pr, 2 * pr + 1)
                psums = []
                for pi, oi in enumerate(ois):
                    psums.append([
                        ps.tile([128, 512], F32, tag=ps_tags[pi][j],
                                name=f"qkps{oi}_{j}", bufs=(1 if pi == 0 else 2))
                        for j in range(4)])
                for wc in range(4):
                    w_cs = []
                    for oi in ois:
                        w_src = wq_d if oi < 4 else wk_d
                        w_c = qkw.tile([128, 8, 128], BF16, tag="w",
                                       name=f"wc{oi}_{wc}")
                        nc.scalar.dma_start(
                            w_c[:],
                            w_src[oi % 4, :, wc * 8:(wc + 1) * 8, :],
                        )
                        w_cs.append(w_c)
                    if not consts_loaded[0]:
                        consts_loaded[0] = True
                        nc.scalar.dma_start(ones_sb[:], ones_d[:, :])
                        nc.scalar.dma_start(tri_sb[:], tri_d[:, :])
                    for kt8 in range(8):
                        kt = wc * 8 + kt8
                        for pi in range(2):
                            for j in range(4):
                                nc.tensor.matmul(
                                    psums[pi][j][:],
                                    w_cs[pi][:, kt8, :],
                                    xt_sb[:, kt, j * 512:(j + 1) * 512],
                                    start=(kt == 0), stop=(kt == KT - 1),
                                )
                for j in range(4):
                    for pi, oi in enumerate(ois):
                        dst = qts if oi < 4 else kts
                        head = oi % 4
                        qt_sb = qkd.tile([128, 512], F32, tag="qt",
                                         name=f"qt{oi}_{j}")
                        nc.scalar.copy(qt_sb[:], psums[pi][j][:])
                        # rotate-half via SBUF->SBUF partition-offset DMA
                        sw = qkd.tile([128, 512], F32, tag="sw",
                                      name=f"sw{oi}_{j}")
                        nc.scalar.dma_start(sw[0:64, :], qt_sb[64:128, :])
                        nc.scalar.dma_start(sw[64:128, :], qt_sb[0:64, :])
                        tmp1 = qkd.tile([128, 512], F32, tag="t1")
                        nc.vector.tensor_mul(
                            tmp1[:], qt_sb[:],
                            fr_sb[:, j * 512:(j + 1) * 512])
                        tmp2 = qkd.tile([128, 512], F32, tag="t2")
                        nc.vector.tensor_mul(
                            tmp2[:], sw[:],
                            fis_sb[:, j * 512:(j + 1) * 512])
                        nc.vector.tensor_add(
                            dst[head][:, j * 512:(j + 1) * 512],
                            tmp1[:], tmp2[:])

        # ============ Section 2: V + attention + AllGather + WO ============
        # right-side pools live until the end; xtp (left) closes after vq3.
        vallp = es.enter_context(tc.tile_pool(name="vallp", bufs=1, side="right"))
        vwp = es.enter_context(tc.tile_pool(name="vwp", bufs=3, side="right"))
        aptB = es.enter_context(tc.tile_pool(name="aptB", bufs=8, side="right"))

        vall = vallp.tile([128, QTILES, 512], BF16, tag="vall")

        cpools = {}
        copy_rr = [0]

        def _scalar_copy(out, in_):
            return nc.scalar.copy(out, in_)

        def copy_engine():
            # rotate PSUM->SBUF copies between vector and scalar
            copy_rr[0] ^= 1
            return nc.vector.tensor_copy if copy_rr[0] else _scalar_copy

        # ---------- V-projection units ----------
        def make_vq_units(vq, xt_src):
            """32 matmul units (one per kt, 4 MMs each) + a final unit doing
            the 4 PSUM->vall copies."""
            psv = [ps.tile([128, 512], F32, tag=f"a{tt}",
                           name=f"vps{vq}_{tt}", bufs=1)
                   for tt in range(4)]
            chunks = {}

            def load_chunk(c):
                wv_c = vwp.tile([128, 2, 512], BF16, tag="wv",
                                name=f"wv{vq}_{c}")
                nc.sync.dma_start(wv_c[:], wv_d[:, 2 * c:2 * c + 2, :])
                chunks[c] = wv_c

            def unit(kt):
                def emit():
                    if kt == 0:
                        load_chunk(0)
                        load_chunk(1)
                        load_chunk(2)
                    elif kt % 2 == 0 and kt // 2 + 2 < KT // 2:
                        load_chunk(kt // 2 + 2)
                    wv_c = chunks[kt // 2]
                    for tt in range(4):
                        nc.tensor.matmul(
                            psv[tt][:],
                            xt_src(kt, tt),
                            wv_c[:, kt % 2, :],
                            start=(kt == 0), stop=(kt == KT - 1),
                        )
                return emit

            units = [unit(kt) for kt in range(KT)]

            def final():
                for tt in range(4):
                    eng = copy_engine()
                    eng(vall[:, vq * 4 + tt, :], psv[tt][:])
            units.append(final)
            return units

        # ---------- attention ----------
        def emit_scores(qr, head):
            """Score matmuls + exp (band tiles first, suffix-trimmed)."""
            kt_h = kts[head]
            q_sl = qts[head][:, qr * 512:(qr + 1) * 512]
            pts = {}
            band = list(range(4 * qr, 4 * qr + 4))
            old = list(range(0, 4 * qr))
            for kt in band + old:
                i = kt - 4 * qr
                c0 = i * 128 if i >= 0 else 0
                ps_t = ps.tile([128, 512], F32, tag="b",
                               name=f"st{qr}_{head}_{kt}", bufs=2)
                nc.tensor.matmul(
                    ps_t[:, c0:], kt_h[:, kt * 128:(kt + 1) * 128],
                    q_sl[:, c0:])
                pool = aptA if kt < 8 else aptB
                pT = pool.tile([128, 512], BF16, tag="pT",
                               name=f"pT{qr}_{head}_{kt}")
                if i >= 0:
                    # diagonal block: exp then 0/1 triangle multiply
                    nc.scalar.activation(
                        pT[:, c0:c0 + 128], ps_t[:, c0:c0 + 128],
                        mybir.ActivationFunctionType.Exp, scale=SCALE)
                    nc.vector.tensor_mul(
                        pT[:, c0:c0 + 128], pT[:, c0:c0 + 128], tri_sb[:])
                    if i < 3:
                        nc.scalar.activation(
                            pT[:, c0 + 128:], ps_t[:, c0 + 128:],
                            mybir.ActivationFunctionType.Exp, scale=SCALE)
                else:
                    nc.scalar.activation(
                        pT[:], ps_t[:],
                        mybir.ActivationFunctionType.Exp, scale=SCALE)
                pts[kt] = pT
            return pts

        def emit_pv(qr, head, pts):
            """PV accumulation + exp-sum partials. Returns the pending
            softmax-finish state (flushed later, pipelined)."""
            nkt = 4 * qr + 4
            band = list(range(4 * qr, 4 * qr + 4))
            old = list(range(0, 4 * qr))
            # exp-sum chains over the full-width old tiles (vector + gpsimd)
            accs = []
            parts = [old[0::2], old[1::2]] if len(old) > 2 else [old]
            engs = [nc.vector, nc.gpsimd]
            for pi, part in enumerate(parts):
                if not part:
                    continue
                eng = engs[pi]
                acc = awk.tile([128, 512], BF16, tag=f"acc{pi}",
                               name=f"acc{pi}_{qr}_{head}", bufs=1)
                if len(part) >= 2:
                    eng.tensor_add(acc[:], pts[part[0]][:], pts[part[1]][:])
                else:
                    eng.tensor_copy(acc[:], pts[part[0]][:])
                for kt in part[2:]:
                    eng.tensor_add(acc[:], acc[:], pts[kt][:])
                accs.append(acc)
            # PV accumulation (old tiles full width, band tiles suffix)
            ps_pv = ps.tile([128, 512], F32, tag="c",
                            name=f"pv{qr}_{head}", bufs=2)
            for kt in range(nkt):
                i = kt - 4 * qr
                c0 = i * 128 if i >= 0 else 0
                nc.tensor.matmul(
                    ps_pv[:, c0:], vall[:, kt, head * 128:(head + 1) * 128],
                    pts[kt][:, c0:],
                    start=(kt == 0), stop=(kt == nkt - 1))
            # denominator: band tiles summed on the PE (suffix ones-matmuls)
            ps_rsb = ps.tile([128, 512], F32, tag="c",
                             name=f"rsb{qr}_{head}", bufs=2)
            for bi, kt in enumerate(band):
                c0 = (kt - 4 * qr) * 128
                nc.tensor.matmul(
                    ps_rsb[:, c0:], ones_sb[:], pts[kt][:, c0:],
                    start=(bi == 0), stop=(bi == 3 and not accs))
            return (qr, head, ps_pv, ps_rsb, accs)

        def emit_finish(pend):
            """Rowsum of the acc chains + reciprocal + divide + agi store."""
            qr, head, ps_pv, ps_rsb, accs = pend
            for ai, acc in enumerate(accs):
                nc.tensor.matmul(ps_rsb[:], ones_sb[:], acc[:],
                                 start=False, stop=(ai == len(accs) - 1))
            rec_bc = awk.tile([128, 512], F32, tag="recb",
                              name=f"rec{qr}_{head}", bufs=1)
            nc.vector.reciprocal(rec_bc[:], ps_rsb[:])
            at_sb = awk.tile([128, 512], BF16, tag="at",
                             name=f"at{qr}_{head}")
            nc.vector.tensor_mul(at_sb[:], ps_pv[:], rec_bc[:])
            nc.gpsimd.dma_start(
                agi[qr][head * 128:(head + 1) * 128, :], at_sb[:])

        def emit_ag(qr):
            nc.gpsimd.collective_compute(
                "AllGather",
                mybir.AluOpType.bypass,
                replica_groups=[list(range(N_CORES))],
                ins=[agi[qr][:].opt()],
                outs=[ago[qr][:].opt()],
            )

        # ---------- WO units ----------
        def make_wo_units(r):
            """32 chunk-closures of 4 MMs each (hh, qtl, dhalf) + an output
            unit per qtl pair (PSUM->SBUF copy + DRAM store)."""
            wo_sb = cpools["wo_sb"]
            ps_os = [ps.tile([128, 512], F32, tag=f"a{qtl}",
                             name=f"wops{r}_{qtl}", bufs=1)
                     for qtl in range(4)]
            atqfs = {}

            def load(hh):
                if hh > 3 or hh in atqfs:
                    return
                atqf = cpools["woa"].tile(
                    [128, 8, 512], BF16, tag="atqf", name=f"atqf{r}_{hh}")
                nc.sync.dma_start(
                    atqf[:],
                    ago[r][hh * 1024:(hh + 1) * 1024, :]
                    .rearrange("(dt p) q -> p dt q", p=128),
                )
                atqfs[hh] = atqf

            def outq(qtl):
                qt = r * 4 + qtl
                o_sb = cpools["woo"].tile([128, 512], F32, tag="osb",
                                          name=f"osb{qt}")
                nc.scalar.copy(o_sb[:], ps_os[qtl][:])
                nc.sync.dma_start(
                    out_d[qt * 128:(qt + 1) * 128, :], o_sb[:])

            units = []
            for hh in range(4):
                for qtl in range(4):
                    for dh in range(2):
                        def chunk(hh=hh, qtl=qtl, dh=dh):
                            if qtl == 0 and dh == 0:
                                load(hh)
                            if qtl == 2 and dh == 0:
                                load(hh + 1)
                            atqf = atqfs[hh]
                            for dt in range(dh * 4, dh * 4 + 4):
                                gdt = hh * 8 + dt
                                nc.tensor.matmul(
                                    ps_os[qtl][:],
                                    atqf[:, dt, qtl * 128:(qtl + 1) * 128],
                                    wo_sb[:, gdt, :],
                                    start=(gdt == 0), stop=(gdt == KT - 1))
                            if hh == 3 and dh == 1:
                                outq(qtl)
                        units.append(chunk)
            return units

        # ---------- emission schedule ----------
        def xt_direct(vq):
            def src(kt, tt):
                base = vq * 512 + tt * 128
                return xt_sb[:, kt, base:base + 128]
            return src

        fillers = []

        def fill(n):
            for _ in range(n):
                if fillers:
                    fillers.pop(0)()

        def drain():
            while fillers:
                fillers.pop(0)()

        pending = [None]

        def attn_head(qr, head, nfill=3, npost=3):
            pts = emit_scores(qr, head)
            if pending[0] is not None:
                emit_finish(pending[0])
            fill(nfill)
            pending[0] = emit_pv(qr, head, pts)
            fill(npost)

        def flush_and_ag(qr):
            emit_finish(pending[0])
            pending[0] = None
            emit_ag(qr)

        # vq0 fully upfront (PV of qr=0 needs tokens 0..511)
        for u in make_vq_units(0, xt_direct(0)):
            u()

        fillers = make_vq_units(1, xt_direct(1))
        for head in range(4):
            attn_head(0, head, nfill=6, npost=2)
        flush_and_ag(0)
        drain()

        fillers = make_vq_units(2, xt_direct(2))
        for head in range(4):
            attn_head(1, head)
        flush_and_ag(1)
        drain()

        fillers = make_vq_units(3, xt_direct(3))
        for head in range(4):
            attn_head(2, head)
        flush_and_ag(2)
        drain()

        # x^T no longer needed: release its 128KB (left side)
        xtp_cm.__exit__(None, None, None)

        # phase-C pools in the freed left space
        wop = es.enter_context(tc.tile_pool(name="wop", bufs=1, side="left"))
        woa = es.enter_context(tc.tile_pool(name="woa", bufs=2, side="left"))
        woo = es.enter_context(tc.tile_pool(name="woo", bufs=2, side="left"))
        wo_sb = wop.tile([128, KT, DSH], BF16, tag="wo")
        cpools["wo_sb"] = wo_sb
        cpools["woa"] = woa
        cpools["woo"] = woo
        for ch in range(8):
            nc.sync.dma_start(
                wo_sb[:, ch * 4:(ch + 1) * 4, :],
                wo_d[:, ch * 4:(ch + 1) * 4, :],
            )

        # attn(3) with wo(0) chunks as fillers; ag(0) completed long ago
        fillers = make_wo_units(0)
        for head in range(4):
            attn_head(3, head)
        flush_and_ag(3)
        drain()           # rest of wo(0) — runs while ag(3) is on the wire
        for u in make_wo_units(1):
            u()
        for u in make_wo_units(2):
            u()
        for u in make_wo_units(3):
            u()

    nc.compile()
    return nc


def _get_program(mode):
    if mode not in _PROGRAMS:
        if mode == "causal":
            _PROGRAMS[mode] = _build_causal()
        else:
            from kernel_baseline import _build_program as _legacy
            _PROGRAMS[mode] = _legacy(mode)
    return _PROGRAMS[mode]


def _prep_inputs(x, wq, wk, wv, wo, freqs_real, freqs_imag, mask):
    """Host-side shard/layout prep. Returns (mode, in_maps)."""
    x = np.asarray(x, dtype=np.float32)
    wq = np.asarray(wq, dtype=np.float32)
    wk = np.asarray(wk, dtype=np.float32)
    wv = np.asarray(wv, dtype=np.float32)
    wo = np.asarray(wo, dtype=np.float32)
    fr = np.asarray(freqs_real, dtype=np.float32)
    fi = np.asarray(freqs_imag, dtype=np.float32)
    m = np.asarray(mask, dtype=np.float32).reshape(S, S)

    causal_ref = np.triu(np.full((S, S), np.float32(-1e9), dtype=np.float32), k=1)
    if np.array_equal(m, causal_ref):
        mode = "causal"
    elif not m.any():
        mode = "nomask"
    else:
        mode = "general"

    if mode != "causal":
        from kernel_baseline import _prep_inputs as _legacy_prep
        return _legacy_prep(x, wq, wk, wv, wo, freqs_real, freqs_imag, mask)

    xT = np.ascontiguousarray(x.reshape(S, D).T)  # [D, S]
    xT_bf = xT.astype(ml_dtypes.bfloat16)

    # evens-first permutation of each head's 128 dims (for RoPE pair layout)
    idx = np.concatenate([np.arange(0, HD, 2), np.arange(1, HD, 2)])
    cols = np.concatenate([h * HD + idx for h in range(32)])
    wq_p = wq[:, cols]
    wk_p = wk[:, cols]

    fr128 = np.ascontiguousarray(np.concatenate([fr.T, fr.T], axis=0))   # [128, S]
    fis128 = np.ascontiguousarray(np.concatenate([-fi.T, fi.T], axis=0))

    onesmat = np.ones((128, 128), dtype=np.float32)
    # multiplicative causal tile mask in [k, q] layout: 1 iff k <= q
    tri01 = (np.arange(128)[:, None] <= np.arange(128)[None, :]).astype(np.float32)

    in_maps = []
    for c in range(N_CORES):
        sl = slice(c * DSH, (c + 1) * DSH)

        def _wtile(a):
            # [D, C] -> [128p, KT, C] matching the SBUF tile layout
            return np.ascontiguousarray(
                a.reshape(KT, 128, a.shape[1]).transpose(1, 0, 2)
            ).astype(ml_dtypes.bfloat16)

        def _whead(a):
            # [D, 512] -> [NH_LOC, 128p, KT, HD]
            return np.ascontiguousarray(np.stack([
                _wtile(a[:, h * HD:(h + 1) * HD]) for h in range(NH_LOC)
            ]))

        im = {
            "xT": xT_bf,
            "wq": _whead(wq_p[:, sl]),
            "wk": _whead(wk_p[:, sl]),
            "wv": _wtile(wv[:, sl]),
            "wo": _wtile(wo[:, sl]),
            "fr128": fr128.astype(ml_dtypes.bfloat16),
            "fis128": fis128.astype(ml_dtypes.bfloat16),
            "onesmat": onesmat.astype(ml_dtypes.bfloat16),
            "tri01": tri01.astype(ml_dtypes.bfloat16),
        }
        in_maps.append(im)
    return mode, in_maps


def kernel(x, wq, wk, wv, wo, cache_k, cache_v, freqs_real, freqs_imag,
           mask, start_pos, **_unused):
    assert int(start_pos) == 0, "kernel hardcodes start_pos=0"
    mode, in_maps = _prep_inputs(x, wq, wk, wv, wo, freqs_real, freqs_imag, mask)
    nc = _get_program(mode)
    res = run_bass_kernel_spmd(nc, in_maps, core_ids=list(range(N_CORES)))
    out = np.concatenate([res.results[c]["out"] for c in range(N_CORES)], axis=1)
    return out.reshape(1, S, D).astype(np.float32)


# revision 18
# speedup vs baseline: 1.0062x; 1.0062x over previous
"""Trainium2 Bass kernel for nn_Attention (dense transformer attention block).

Full causal attention: QKV projection + RoPE + softmax(QK^T/sqrt(d) + mask)V + WO,
bsz=1, seqlen=2048, dim=4096, 32 heads x head_dim 128, fp32 I/O.

Sharding: tensor-parallel across heads on 8 NeuronCores. Core c owns heads
4c..4c+3 (wq/wk/wv output columns, attention) and wo output columns
512c..512c+512 (after an AllGather of the per-core attn^T shard along the
head axis). Host concatenates the 8 output column shards.

Schedule (causal fast path): the PE is the bottleneck (board power throttle
caps it at ~1.95GHz), so the emission order is built to keep it gap-free:
  - Q kept resident in SBUF (no DRAM spill round-trip).
  - V-projection is emitted as small filler chunks between the score and
    PV matmuls of each attention head, covering the exp-chain latency.
  - Causal diagonal-band tiles compute only the unmasked column suffix;
    masking is a multiplicative 0/1 triangle on the diagonal block.
  - wo(1)/wo(2) are held back until after the last AllGather is issued so
    the collective hides under them; the tail is a single wo pass.
"""

import contextlib

import ml_dtypes
import numpy as np

import concourse.bacc as bacc
import concourse.mybir as mybir
import concourse.tile as tile
from concourse.bass_utils import run_bass_kernel_spmd

# Problem constants (hardcoded per contract)
N_CORES = 8
S = 2048              # sequence length
D = 4096              # model dim
HD = 128              # head dim
NH_LOC = 4            # heads per core
DSH = 512             # per-core shard width (NH_LOC * HD)
KT = D // 128         # 32 contraction tiles over model dim
QTILES = S // 128     # 16 token tiles
QRANGES = S // 512    # 4 query ranges of 512
SCALE = float(1.0 / np.sqrt(HD))

F32 = mybir.dt.float32
F32R = mybir.dt.float32r
BF16 = mybir.dt.bfloat16

_PROGRAMS = {}


def _build_causal():
    nc = bacc.Bacc("TRN2", target_bir_lowering=False, debug=False,
                   num_devices=N_CORES)

    # ---- external inputs (per core) ----
    xT_d = nc.dram_tensor("xT", [D, S], BF16, kind="ExternalInput")
    wq_d = nc.dram_tensor("wq", [NH_LOC, 128, KT, HD], BF16, kind="ExternalInput")
    wk_d = nc.dram_tensor("wk", [NH_LOC, 128, KT, HD], BF16, kind="ExternalInput")
    wv_d = nc.dram_tensor("wv", [128, KT, DSH], BF16, kind="ExternalInput")
    wo_d = nc.dram_tensor("wo", [128, KT, DSH], BF16, kind="ExternalInput")
    fr_d = nc.dram_tensor("fr128", [128, S], BF16, kind="ExternalInput")
    fis_d = nc.dram_tensor("fis128", [128, S], BF16, kind="ExternalInput")
    ones_d = nc.dram_tensor("onesmat", [128, 128], BF16, kind="ExternalInput")
    tri_d = nc.dram_tensor("tri01", [128, 128], BF16, kind="ExternalInput")
    out_d = nc.dram_tensor("out", [S, DSH], F32, kind="ExternalOutput")

    with tile.TileContext(nc) as tc, contextlib.ExitStack() as es:
        # ---- persistent pools (left side) ----
        cns = es.enter_context(tc.tile_pool(name="consts", bufs=1, side="left"))
        dram = es.enter_context(tc.tile_pool(name="dram", bufs=1, space="DRAM"))
        akv = es.enter_context(tc.tile_pool(name="akv", bufs=1, side="left"))
        aptA = es.enter_context(tc.tile_pool(name="aptA", bufs=8, side="left"))
        awk = es.enter_context(tc.tile_pool(name="awk", bufs=2, side="left"))
        ps = es.enter_context(tc.tile_pool(name="ps", bufs=1, space="PSUM"))

        agi = [dram.tile([DSH, 512], BF16, name=f"agi{r}") for r in range(4)]
        ago = [dram.tile([D, 512], BF16, addr_space="Shared", name=f"ago{r}")
               for r in range(4)]

        ones_sb = cns.tile([128, 128], BF16, tag="ones")
        tri_sb = cns.tile([128, 128], BF16, tag="tri")

        kts = [akv.tile([128, S], BF16, tag=f"kth{h}", name=f"kth{h}")
               for h in range(NH_LOC)]
        qts = [akv.tile([128, S], BF16, tag=f"qth{h}", name=f"qth{h}")
               for h in range(NH_LOC)]

        # ---- transient pool: x^T fully resident (left, 128KB/part) ----
        xtp_cm = tc.tile_pool(name="xtp", bufs=1, side="left")
        xtp = xtp_cm.__enter__()
        xt_sb = xtp.tile([128, KT, S], BF16, tag="xt")

        # ================= Section 1: Q/K projections + RoPE =================
        with (
            tc.tile_pool(name="qkc", bufs=1, side="left") as qkc,
            tc.tile_pool(name="qkw", bufs=4, side="left") as qkw,
            tc.tile_pool(name="qkd", bufs=2, side="left") as qkd,
        ):
            fr_sb = qkc.tile([128, S], BF16, tag="fr")
            fis_sb = qkc.tile([128, S], BF16, tag="fis")

            # first kt tile split into 4 column quarters on 4 queues so the
            # very first matmul's moving operand lands as early as possible
            for qi, eng in enumerate((nc.sync, nc.scalar, nc.gpsimd, nc.sync)):
                eng.dma_start(
                    xt_sb[:, 0, qi * 512:(qi + 1) * 512],
                    xT_d[0:128, qi * 512:(qi + 1) * 512],
                )
            xt_chunks = [(1, 2), (2, 4), (4, 7), (7, 11), (11, 16),
                         (16, 22), (22, 28), (28, 32)]
            for ch, (k0, k1) in enumerate(xt_chunks):
                nc.sync.dma_start(
                    xt_sb[:, k0:k1, :],
                    xT_d[k0 * 128:k1 * 128, :]
                    .rearrange("(kt p) s -> p kt s", p=128),
                )
                if ch == 0:
                    nc.gpsimd.dma_start(fr_sb[:], fr_d[:, :])
                    nc.gpsimd.dma_start(fis_sb[:], fis_d[:, :])

            # oi pairs in kt-major order: halves the x-consumption rate so
            # the first pass never outruns the x DMA stream.
            ps_tags = [[f"a{j}" for j in range(4)],
                       ["b", "b", "c", "c"]]
            consts_loaded = [False]
            for pr in range(4):
                ois = (2 * pr, 2 * pr + 1)
                psums = []
                for pi, oi in enumerate(ois):
                    psums.append([
                        ps.tile([128, 512], F32, tag=ps_tags[pi][j],
                                name=f"qkps{oi}_{j}", bufs=(1 if pi == 0 else 2))
                        for j in range(4)])
                for wc in range(4):
                    w_cs = []
                    for oi in ois:
                        w_src = wq_d if oi < 4 else wk_d
                        w_c = qkw.tile([128, 8, 128], BF16, tag="w",
                                       name=f"wc{oi}_{wc}")
                        nc.scalar.dma_start(
                            w_c[:],
                            w_src[oi % 4, :, wc * 8:(wc + 1) * 8, :],
                        )
                        w_cs.append(w_c)
                    if not consts_loaded[0]:
                        consts_loaded[0] = True
                        nc.scalar.dma_start(ones_sb[:], ones_d[:, :])
                        nc.scalar.dma_start(tri_sb[:], tri_d[:, :])
                    for kt8 in range(8):
                        kt = wc * 8 + kt8
                        for pi in range(2):
                            for j in range(4):
                                nc.tensor.matmul(
                                    psums[pi][j][:],
                                    w_cs[pi][:, kt8, :],
                                    xt_sb[:, kt, j * 512:(j + 1) * 512],
                                    start=(kt == 0), stop=(kt == KT - 1),
                                )
                for j in range(4):
                    for pi, oi in enumerate(ois):
                        dst = qts if oi < 4 else kts
                        head = oi % 4
                        qt_sb = qkd.tile([128, 512], F32, tag="qt",
                                         name=f"qt{oi}_{j}")
                        nc.scalar.copy(qt_sb[:], psums[pi][j][:])
                        # rotate-half via SBUF->SBUF partition-offset DMA
                        sw = qkd.tile([128, 512], F32, tag="sw",
                                      name=f"sw{oi}_{j}")
                        nc.sync.dma_start(sw[0:64, :], qt_sb[64:128, :])
                        nc.sync.dma_start(sw[64:128, :], qt_sb[0:64, :])
                        tmp1 = qkd.tile([128, 512], F32, tag="t1")
                        nc.vector.tensor_mul(
                            tmp1[:], qt_sb[:],
                            fr_sb[:, j * 512:(j + 1) * 512])
                        tmp2 = qkd.tile([128, 512], F32, tag="t2")
                        nc.gpsimd.tensor_mul(
                            tmp2[:], sw[:],
                            fis_sb[:, j * 512:(j + 1) * 512])
                        nc.vector.tensor_add(
                            dst[head][:, j * 512:(j + 1) * 512],
                            tmp1[:], tmp2[:])

        # ============ Section 2: V + attention + AllGather + WO ============
        # right-side pools live until the end; xtp (left) closes after vq3.
        vallp = es.enter_context(tc.tile_pool(name="vallp", bufs=1, side="right"))
        vwp = es.enter_context(tc.tile_pool(name="vwp", bufs=3, side="right"))
        aptB = es.enter_context(tc.tile_pool(name="aptB", bufs=8, side="right"))

        vall = vallp.tile([128, QTILES, 512], BF16, tag="vall")

        cpools = {}
        copy_rr = [0]

        def _scalar_copy(out, in_):
            return nc.scalar.copy(out, in_)

        def copy_engine():
            # rotate PSUM->SBUF copies between vector and scalar
            copy_rr[0] ^= 1
            return nc.vector.tensor_copy if copy_rr[0] else _scalar_copy

        # ---------- V-projection units ----------
        def make_vq_units(vq, xt_src):
            """32 matmul units (one per kt, 4 MMs each) + a final unit doing
            the 4 PSUM->vall copies."""
            psv = [ps.tile([128, 512], F32, tag=f"a{tt}",
                           name=f"vps{vq}_{tt}", bufs=1)
                   for tt in range(4)]
            chunks = {}

            def load_chunk(c):
                wv_c = vwp.tile([128, 2, 512], BF16, tag="wv",
                                name=f"wv{vq}_{c}")
                nc.sync.dma_start(wv_c[:], wv_d[:, 2 * c:2 * c + 2, :])
                chunks[c] = wv_c

            def unit(kt):
                def emit():
                    if kt == 0:
                        load_chunk(0)
                        load_chunk(1)
                        load_chunk(2)
                    elif kt % 2 == 0 and kt // 2 + 2 < KT // 2:
                        load_chunk(kt // 2 + 2)
                    wv_c = chunks[kt // 2]
                    for tt in range(4):
                        nc.tensor.matmul(
                            psv[tt][:],
                            xt_src(kt, tt),
                            wv_c[:, kt % 2, :],
                            start=(kt == 0), stop=(kt == KT - 1),
                        )
                return emit

            units = [unit(kt) for kt in range(KT)]

            def final():
                for tt in range(4):
                    eng = copy_engine()
                    eng(vall[:, vq * 4 + tt, :], psv[tt][:])
            units.append(final)
            return units

        # ---------- attention ----------
        def emit_scores(qr, head):
            """Score matmuls + exp (band tiles first, suffix-trimmed)."""
            kt_h = kts[head]
            q_sl = qts[head][:, qr * 512:(qr + 1) * 512]
            pts = {}
            band = list(range(4 * qr, 4 * qr + 4))
            old = list(range(0, 4 * qr))
            for kt in band + old:
                i = kt - 4 * qr
                c0 = i * 128 if i >= 0 else 0
                ps_t = ps.tile([128, 512], F32, tag="b",
                               name=f"st{qr}_{head}_{kt}", bufs=2)
                nc.tensor.matmul(
                    ps_t[:, c0:], kt_h[:, kt * 128:(kt + 1) * 128],
                    q_sl[:, c0:])
                pool = aptA if kt < 8 else aptB
                pT = pool.tile([128, 512], BF16, tag="pT",
                               name=f"pT{qr}_{head}_{kt}")
                if i >= 0:
                    # diagonal block: exp then 0/1 triangle multiply
                    nc.scalar.activation(
                        pT[:, c0:c0 + 128], ps_t[:, c0:c0 + 128],
                        mybir.ActivationFunctionType.Exp, scale=SCALE)
                    nc.vector.tensor_mul(
                        pT[:, c0:c0 + 128], pT[:, c0:c0 + 128], tri_sb[:])
                    if i < 3:
                        nc.scalar.activation(
                            pT[:, c0 + 128:], ps_t[:, c0 + 128:],
                            mybir.ActivationFunctionType.Exp, scale=SCALE)
                else:
                    nc.scalar.activation(
                        pT[:], ps_t[:],
                        mybir.ActivationFunctionType.Exp, scale=SCALE)
                pts[kt] = pT
            return pts

        def emit_pv(qr, head, pts):
            """PV accumulation + exp-sum partials. Returns the pending
            softmax-finish state (flushed later, pipelined)."""
            nkt = 4 * qr + 4
            band = list(range(4 * qr, 4 * qr + 4))
            old = list(range(0, 4 * qr))
            # exp-sum chains over the full-width old tiles (vector + gpsimd)
            accs = []
            parts = [old[0::2], old[1::2]] if len(old) > 2 else [old]
            engs = [nc.vector, nc.gpsimd]
            for pi, part in enumerate(parts):
                if not part:
                    continue
                eng = engs[pi]
                acc = awk.tile([128, 512], BF16, tag=f"acc{pi}",
                               name=f"acc{pi}_{qr}_{head}", bufs=1)
                if len(part) >= 2:
                    eng.tensor_add(acc[:], pts[part[0]][:], pts[part[1]][:])
                else:
                    eng.tensor_copy(acc[:], pts[part[0]][:])
                for kt in part[2:]:
                    eng.tensor_add(acc[:], acc[:], pts[kt][:])
                accs.append(acc)
            # PV accumulation (old tiles full width, band tiles suffix)
            ps_pv = ps.tile([128, 512], F32, tag="c",
                            name=f"pv{qr}_{head}", bufs=2)
            for kt in range(nkt):
                i = kt - 4 * qr
                c0 = i * 128 if i >= 0 else 0
                nc.tensor.matmul(
                    ps_pv[:, c0:], vall[:, kt, head * 128:(head + 1) * 128],
                    pts[kt][:, c0:],
                    start=(kt == 0), stop=(kt == nkt - 1))
            # denominator: band tiles summed on the PE (suffix ones-matmuls)
            ps_rsb = ps.tile([128, 512], F32, tag="c",
                             name=f"rsb{qr}_{head}", bufs=2)
            for bi, kt in enumerate(band):
                c0 = (kt - 4 * qr) * 128
                nc.tensor.matmul(
                    ps_rsb[:, c0:], ones_sb[:], pts[kt][:, c0:],
                    start=(bi == 0), stop=(bi == 3 and not accs))
            return (qr, head, ps_pv, ps_rsb, accs)

        def emit_finish(pend):
            """Rowsum of the acc chains + reciprocal + divide + agi store."""
            qr, head, ps_pv, ps_rsb, accs = pend
            for ai, acc in enumerate(accs):
                nc.tensor.matmul(ps_rsb[:], ones_sb[:], acc[:],
                                 start=False, stop=(ai == len(accs) - 1))
            rec_bc = awk.tile([128, 512], F32, tag="recb",
                              name=f"rec{qr}_{head}", bufs=1)
            nc.vector.reciprocal(rec_bc[:], ps_rsb[:])
            at_sb = awk.tile([128, 512], BF16, tag="at",
                             name=f"at{qr}_{head}")
            nc.vector.tensor_mul(at_sb[:], ps_pv[:], rec_bc[:])
            nc.gpsimd.dma_start(
                agi[qr][head * 128:(head + 1) * 128, :], at_sb[:])

        def emit_ag(qr):
            nc.gpsimd.collective_compute(
                "AllGather",
                mybir.AluOpType.bypass,
                replica_groups=[list(range(N_CORES))],
                ins=[agi[qr][:].opt()],
                outs=[ago[qr][:].opt()],
            )

        # ---------- WO units ----------
        def make_wo_units(r):
            """32 chunk-closures of 4 MMs each (hh, qtl, dhalf) + an output
            unit per qtl pair (PSUM->SBUF copy + DRAM store)."""
            wo_sb = cpools["wo_sb"]
            ps_os = [ps.tile([128, 512], F32, tag=f"a{qtl}",
                             name=f"wops{r}_{qtl}", bufs=1)
                     for qtl in range(4)]
            atqfs = {}

            def load(hh):
                if hh > 3 or hh in atqfs:
                    return
                atqf = cpools["woa"].tile(
                    [128, 8, 512], BF16, tag="atqf", name=f"atqf{r}_{hh}")
                nc.sync.dma_start(
                    atqf[:],
                    ago[r][hh * 1024:(hh + 1) * 1024, :]
                    .rearrange("(dt p) q -> p dt q", p=128),
                )
                atqfs[hh] = atqf

            def outq(qtl):
                qt = r * 4 + qtl
                o_sb = cpools["woo"].tile([128, 512], F32, tag="osb",
                                          name=f"osb{qt}")
                nc.scalar.copy(o_sb[:], ps_os[qtl][:])
                nc.sync.dma_start(
                    out_d[qt * 128:(qt + 1) * 128, :], o_sb[:])

            units = []
            for hh in range(4):
                for qtl in range(4):
                    for dh in range(2):
                        def chunk(hh=hh, qtl=qtl, dh=dh):
                            if qtl == 0 and dh == 0:
                                load(hh)
                            if qtl == 2 and dh == 0:
                                load(hh + 1)
                            atqf = atqfs[hh]
                            for dt in range(dh * 4, dh * 4 + 4):
                                gdt = hh * 8 + dt
                                nc.tensor.matmul(
                                    ps_os[qtl][:],
                                    atqf[:, dt, qtl * 128:(qtl + 1) * 128],
                                    wo_sb[:, gdt, :],
                                    start=(gdt == 0), stop=(gdt == KT - 1))
                            if hh == 3 and dh == 1:
                                outq(qtl)
                        units.append(chunk)
            return units

        # ---------- emission schedule ----------
        def xt_direct(vq):
            def src(kt, tt):
                base = vq * 512 + tt * 128
                return xt_sb[:, kt, base:base + 128]
            return src

        fillers = []

        def fill(n):
            for _ in range(n):
                if fillers:
                    fillers.pop(0)()

        def drain():
            while fillers:
                fillers.pop(0)()

        pending = [None]

        def attn_head(qr, head, nfill=3, npost=3):
            pts = emit_scores(qr, head)
            if pending[0] is not None:
                emit_finish(pending[0])
            fill(nfill)
            pending[0] = emit_pv(qr, head, pts)
            fill(npost)

        def flush_and_ag(qr):
            emit_finish(pending[0])
            pending[0] = None
            emit_ag(qr)

        # vq0 fully upfront (PV of qr=0 needs tokens 0..511)
        for u in make_vq_units(0, xt_direct(0)):
            u()

        fillers = make_vq_units(1, xt_direct(1))
        for head in range(4):
            attn_head(0, head, nfill=6, npost=2)
        flush_and_ag(0)
        drain()

        fillers = make_vq_units(2, xt_direct(2))
        for head in range(4):
            attn_head(1, head)
        flush_and_ag(1)
        drain()

        fillers = make_vq_units(3, xt_direct(3))
        for head in range(4):
            attn_head(2, head)
        flush_and_ag(2)
        drain()

        # x^T no longer needed: release its 128KB (left side)
        xtp_cm.__exit__(None, None, None)

        # phase-C pools in the freed left space
        wop = es.enter_context(tc.tile_pool(name="wop", bufs=1, side="left"))
        woa = es.enter_context(tc.tile_pool(name="woa", bufs=2, side="left"))
        woo = es.enter_context(tc.tile_pool(name="woo", bufs=2, side="left"))
        wo_sb = wop.tile([128, KT, DSH], BF16, tag="wo")
        cpools["wo_sb"] = wo_sb
        cpools["woa"] = woa
        cpools["woo"] = woo
        for ch in range(8):
            nc.sync.dma_start(
                wo_sb[:, ch * 4:(ch + 1) * 4, :],
                wo_d[:, ch * 4:(ch + 1) * 4, :],
            )

        # attn(3) with wo(0) chunks as fillers; ag(0) completed long ago
        fillers = make_wo_units(0)
        for head in range(4):
            attn_head(3, head)
        flush_and_ag(3)
        drain()           # rest of wo(0) — runs while ag(3) is on the wire
        for u in make_wo_units(1):
            u()
        for u in make_wo_units(2):
            u()
        for u in make_wo_units(3):
            u()

    nc.compile()
    return nc


def _get_program(mode):
    if mode not in _PROGRAMS:
        if mode == "causal":
            _PROGRAMS[mode] = _build_causal()
        else:
            from kernel_baseline import _build_program as _legacy
            _PROGRAMS[mode] = _legacy(mode)
    return _PROGRAMS[mode]


def _prep_inputs(x, wq, wk, wv, wo, freqs_real, freqs_imag, mask):
    """Host-side shard/layout prep. Returns (mode, in_maps)."""
    x = np.asarray(x, dtype=np.float32)
    wq = np.asarray(wq, dtype=np.float32)
    wk = np.asarray(wk, dtype=np.float32)
    wv = np.asarray(wv, dtype=np.float32)
    wo = np.asarray(wo, dtype=np.float32)
    fr = np.asarray(freqs_real, dtype=np.float32)
    fi = np.asarray(freqs_imag, dtype=np.float32)
    m = np.asarray(mask, dtype=np.float32).reshape(S, S)

    causal_ref = np.triu(np.full((S, S), np.float32(-1e9), dtype=np.float32), k=1)
    if np.array_equal(m, causal_ref):
        mode = "causal"
    elif not m.any():
        mode = "nomask"
    else:
        mode = "general"

    if mode != "causal":
        from kernel_baseline import _prep_inputs as _legacy_prep
        return _legacy_prep(x, wq, wk, wv, wo, freqs_real, freqs_imag, mask)

    xT = np.ascontiguousarray(x.reshape(S, D).T)  # [D, S]
    xT_bf = xT.astype(ml_dtypes.bfloat16)

    # evens-first permutation of each head's 128 dims (for RoPE pair layout)
    idx = np.concatenate([np.arange(0, HD, 2), np.arange(1, HD, 2)])
    cols = np.concatenate([h * HD + idx for h in range(32)])
    wq_p = wq[:, cols]
    wk_p = wk[:, cols]

    fr128 = np.ascontiguousarray(np.concatenate([fr.T, fr.T], axis=0))   # [128, S]
    fis128 = np.ascontiguousarray(np.concatenate([-fi.T, fi.T], axis=0))

    onesmat = np.ones((128, 128), dtype=np.float32)
    # multiplicative causal tile mask in [k, q] layout: 1 iff k <= q
    tri01 = (np.arange(128)[:, None] <= np.arange(128)[None, :]).astype(np.float32)

    in_maps = []
    for c in range(N_CORES):
        sl = slice(c * DSH, (c + 1) * DSH)

        def _wtile(a):
            # [D, C] -> [128p, KT, C] matching the SBUF tile layout
            return np.ascontiguousarray(
                a.reshape(KT, 128, a.shape[1]).transpose(1, 0, 2)
            ).astype(ml_dtypes.bfloat16)

        def _whead(a):
            # [D, 512] -> [NH_LOC, 128p, KT, HD]
            return np.ascontiguousarray(np.stack([
                _wtile(a[:, h * HD:(h + 1) * HD]) for h in range(NH_LOC)
            ]))

        im = {
            "xT": xT_bf,
            "wq": _whead(wq_p[:, sl]),
            "wk": _whead(wk_p[:, sl]),
            "wv": _wtile(wv[:, sl]),
            "wo": _wtile(wo[:, sl]),
            "fr128": fr128.astype(ml_dtypes.bfloat16),
            "fis128": fis128.astype(ml_dtypes.bfloat16),
            "onesmat": onesmat.astype(ml_dtypes.bfloat16),
            "tri01": tri01.astype(ml_dtypes.bfloat16),
        }
        in_maps.append(im)
    return mode, in_maps


def kernel(x, wq, wk, wv, wo, cache_k, cache_v, freqs_real, freqs_imag,
           mask, start_pos, **_unused):
    assert int(start_pos) == 0, "kernel hardcodes start_pos=0"
    mode, in_maps = _prep_inputs(x, wq, wk, wv, wo, freqs_real, freqs_imag, mask)
    nc = _get_program(mode)
    res = run_bass_kernel_spmd(nc, in_maps, core_ids=list(range(N_CORES)))
    out = np.concatenate([res.results[c]["out"] for c in range(N_CORES)], axis=1)
    return out.reshape(1, S, D).astype(np.float32)


# revision 19
# speedup vs baseline: 1.0110x; 1.0047x over previous
"""Trainium2 Bass kernel for nn_Attention (dense transformer attention block).

Full causal attention: QKV projection + RoPE + softmax(QK^T/sqrt(d) + mask)V + WO,
bsz=1, seqlen=2048, dim=4096, 32 heads x head_dim 128, fp32 I/O.

Sharding: tensor-parallel across heads on 8 NeuronCores. Core c owns heads
4c..4c+3 (wq/wk/wv output columns, attention) and wo output columns
512c..512c+512 (after an AllGather of the per-core attn^T shard along the
head axis). Host concatenates the 8 output column shards.

Schedule (causal fast path): the PE is the bottleneck (board power throttle
caps it at ~1.95GHz), so the emission order is built to keep it gap-free:
  - Q kept resident in SBUF (no DRAM spill round-trip).
  - V-projection is emitted as small filler chunks between the score and
    PV matmuls of each attention head, covering the exp-chain latency.
  - Causal diagonal-band tiles compute only the unmasked column suffix;
    masking is a multiplicative 0/1 triangle on the diagonal block.
  - wo(1)/wo(2) are held back until after the last AllGather is issued so
    the collective hides under them; the tail is a single wo pass.
"""

import contextlib

import ml_dtypes
import numpy as np

import concourse.bacc as bacc
import concourse.mybir as mybir
import concourse.tile as tile
from concourse.bass_utils import run_bass_kernel_spmd

# Problem constants (hardcoded per contract)
N_CORES = 8
S = 2048              # sequence length
D = 4096              # model dim
HD = 128              # head dim
NH_LOC = 4            # heads per core
DSH = 512             # per-core shard width (NH_LOC * HD)
KT = D // 128         # 32 contraction tiles over model dim
QTILES = S // 128     # 16 token tiles
QRANGES = S // 512    # 4 query ranges of 512
SCALE = float(1.0 / np.sqrt(HD))

F32 = mybir.dt.float32
F32R = mybir.dt.float32r
BF16 = mybir.dt.bfloat16

_PROGRAMS = {}


def _build_causal():
    nc = bacc.Bacc("TRN2", target_bir_lowering=False, debug=False,
                   num_devices=N_CORES)

    # ---- external inputs (per core) ----
    xT_d = nc.dram_tensor("xT", [D, S], BF16, kind="ExternalInput")
    wq_d = nc.dram_tensor("wq", [NH_LOC, 128, KT, HD], BF16, kind="ExternalInput")
    wk_d = nc.dram_tensor("wk", [NH_LOC, 128, KT, HD], BF16, kind="ExternalInput")
    wv_d = nc.dram_tensor("wv", [128, KT, DSH], BF16, kind="ExternalInput")
    wo_d = nc.dram_tensor("wo", [128, KT, DSH], BF16, kind="ExternalInput")
    perm_d = nc.dram_tensor("perm", [128, 128], F32R, kind="ExternalInput")
    fr_d = nc.dram_tensor("fr128", [128, S], BF16, kind="ExternalInput")
    fis_d = nc.dram_tensor("fis128", [128, S], BF16, kind="ExternalInput")
    ones_d = nc.dram_tensor("onesmat", [128, 128], BF16, kind="ExternalInput")
    tri_d = nc.dram_tensor("tri01", [128, 128], BF16, kind="ExternalInput")
    out_d = nc.dram_tensor("out", [S, DSH], F32, kind="ExternalOutput")

    with tile.TileContext(nc) as tc, contextlib.ExitStack() as es:
        # ---- persistent pools (left side) ----
        cns = es.enter_context(tc.tile_pool(name="consts", bufs=1, side="left"))
        dram = es.enter_context(tc.tile_pool(name="dram", bufs=1, space="DRAM"))
        akv = es.enter_context(tc.tile_pool(name="akv", bufs=1, side="left"))
        aptA = es.enter_context(tc.tile_pool(name="aptA", bufs=8, side="left"))
        awk = es.enter_context(tc.tile_pool(name="awk", bufs=2, side="left"))
        ps = es.enter_context(tc.tile_pool(name="ps", bufs=1, space="PSUM"))

        agi = [dram.tile([DSH, 512], BF16, name=f"agi{r}") for r in range(4)]
        ago = [dram.tile([D, 512], BF16, addr_space="Shared", name=f"ago{r}")
               for r in range(4)]

        perm_sb = cns.tile([128, 128], F32R, tag="perm")
        ones_sb = cns.tile([128, 128], BF16, tag="ones")
        tri_sb = cns.tile([128, 128], BF16, tag="tri")

        kts = [akv.tile([128, S], BF16, tag=f"kth{h}", name=f"kth{h}")
               for h in range(NH_LOC)]
        qts = [akv.tile([128, S], BF16, tag=f"qth{h}", name=f"qth{h}")
               for h in range(NH_LOC)]

        # ---- transient pool: x^T fully resident (left, 128KB/part) ----
        xtp_cm = tc.tile_pool(name="xtp", bufs=1, side="left")
        xtp = xtp_cm.__enter__()
        xt_sb = xtp.tile([128, KT, S], BF16, tag="xt")

        # ================= Section 1: Q/K projections + RoPE =================
        with (
            tc.tile_pool(name="qkc", bufs=1, side="left") as qkc,
            tc.tile_pool(name="qkw", bufs=4, side="left") as qkw,
            tc.tile_pool(name="qkd", bufs=2, side="left") as qkd,
        ):
            fr_sb = qkc.tile([128, S], BF16, tag="fr")
            fis_sb = qkc.tile([128, S], BF16, tag="fis")

            # first kt tile split into 4 column quarters on 4 queues so the
            # very first matmul's moving operand lands as early as possible
            for qi, eng in enumerate((nc.sync, nc.scalar, nc.gpsimd, nc.sync)):
                eng.dma_start(
                    xt_sb[:, 0, qi * 512:(qi + 1) * 512],
                    xT_d[0:128, qi * 512:(qi + 1) * 512],
                )
            xt_chunks = [(1, 2), (2, 4), (4, 7), (7, 11), (11, 16),
                         (16, 22), (22, 28), (28, 32)]
            for ch, (k0, k1) in enumerate(xt_chunks):
                nc.sync.dma_start(
                    xt_sb[:, k0:k1, :],
                    xT_d[k0 * 128:k1 * 128, :]
                    .rearrange("(kt p) s -> p kt s", p=128),
                )
                if ch == 0:
                    nc.gpsimd.dma_start(fr_sb[:], fr_d[:, :])
                    nc.gpsimd.dma_start(fis_sb[:], fis_d[:, :])

            # oi pairs in kt-major order: halves the x-consumption rate so
            # the first pass never outruns the x DMA stream.
            ps_tags = [[f"a{j}" for j in range(4)],
                       ["b", "b", "c", "c"]]
            consts_loaded = [False]
            for pr in range(4):
                ois = (2 * pr, 2 * pr + 1)
                psums = []
                for pi, oi in enumerate(ois):
                    psums.append([
                        ps.tile([128, 512], F32, tag=ps_tags[pi][j],
                                name=f"qkps{oi}_{j}", bufs=(1 if pi == 0 else 2))
                        for j in range(4)])
                for wc in range(4):
                    w_cs = []
                    for oi in ois:
                        w_src = wq_d if oi < 4 else wk_d
                        w_c = qkw.tile([128, 8, 128], BF16, tag="w",
                                       name=f"wc{oi}_{wc}")
                        nc.scalar.dma_start(
                            w_c[:],
                            w_src[oi % 4, :, wc * 8:(wc + 1) * 8, :],
                        )
                        w_cs.append(w_c)
                    if not consts_loaded[0]:
                        consts_loaded[0] = True
                        nc.scalar.dma_start(perm_sb[:], perm_d[:, :])
                        nc.scalar.dma_start(ones_sb[:], ones_d[:, :])
                        nc.scalar.dma_start(tri_sb[:], tri_d[:, :])
                    for kt8 in range(8):
                        kt = wc * 8 + kt8
                        for pi in range(2):
                            for j in range(4):
                                nc.tensor.matmul(
                                    psums[pi][j][:],
                                    w_cs[pi][:, kt8, :],
                                    xt_sb[:, kt, j * 512:(j + 1) * 512],
                                    start=(kt == 0), stop=(kt == KT - 1),
                                )
                for j in range(4):
                    for pi, oi in enumerate(ois):
                        dst = qts if oi < 4 else kts
                        head = oi % 4
                        qt_sb = qkd.tile([128, 512], F32R, tag="qt",
                                         name=f"qt{oi}_{j}")
                        nc.scalar.copy(qt_sb[:], psums[pi][j][:])
                        # reuse the just-freed PSUM bank for the swap matmul
                        swap_ps = ps.tile([128, 512], F32, tag=ps_tags[pi][j],
                                          name=f"swap{oi}_{j}",
                                          bufs=(1 if pi == 0 else 2))
                        nc.tensor.matmul(swap_ps[:], perm_sb[:], qt_sb[:])
                        tmp1 = qkd.tile([128, 512], F32, tag="t1")
                        nc.vector.tensor_mul(
                            tmp1[:], qt_sb[:],
                            fr_sb[:, j * 512:(j + 1) * 512])
                        tmp2 = qkd.tile([128, 512], F32, tag="t2")
                        nc.vector.tensor_mul(
                            tmp2[:], swap_ps[:],
                            fis_sb[:, j * 512:(j + 1) * 512])
                        nc.vector.tensor_add(
                            dst[head][:, j * 512:(j + 1) * 512],
                            tmp1[:], tmp2[:])

        # ============ Section 2: V + attention + AllGather + WO ============
        # right-side pools live until the end; xtp (left) closes after vq3.
        vallp = es.enter_context(tc.tile_pool(name="vallp", bufs=1, side="right"))
        vwp = es.enter_context(tc.tile_pool(name="vwp", bufs=3, side="right"))
        aptB = es.enter_context(tc.tile_pool(name="aptB", bufs=8, side="right"))

        vall = vallp.tile([128, QTILES, 512], BF16, tag="vall")

        cpools = {}
        copy_rr = [0]

        def _scalar_copy(out, in_):
            return nc.scalar.copy(out, in_)

        def copy_engine():
            # rotate PSUM->SBUF copies between vector and scalar
            copy_rr[0] ^= 1
            return nc.vector.tensor_copy if copy_rr[0] else _scalar_copy

        # ---------- V-projection units ----------
        def make_vq_units(vq, xt_src):
            """32 matmul units (one per kt, 4 MMs each) + a final unit doing
            the 4 PSUM->vall copies."""
            psv = [ps.tile([128, 512], F32, tag=f"a{tt}",
                           name=f"vps{vq}_{tt}", bufs=1)
                   for tt in range(4)]
            chunks = {}

            def load_chunk(c):
                wv_c = vwp.tile([128, 2, 512], BF16, tag="wv",
                                name=f"wv{vq}_{c}")
                nc.sync.dma_start(wv_c[:], wv_d[:, 2 * c:2 * c + 2, :])
                chunks[c] = wv_c

            def unit(kt):
                def emit():
                    if kt == 0:
                        load_chunk(0)
                        load_chunk(1)
                        load_chunk(2)
                    elif kt % 2 == 0 and kt // 2 + 2 < KT // 2:
                        load_chunk(kt // 2 + 2)
                    wv_c = chunks[kt // 2]
                    for tt in range(4):
                        nc.tensor.matmul(
                            psv[tt][:],
                            xt_src(kt, tt),
                            wv_c[:, kt % 2, :],
                            start=(kt == 0), stop=(kt == KT - 1),
                        )
                return emit

            units = [unit(kt) for kt in range(KT)]

            def final():
                for tt in range(4):
                    eng = copy_engine()
                    eng(vall[:, vq * 4 + tt, :], psv[tt][:])
            units.append(final)
            return units

        # ---------- attention ----------
        def emit_scores(qr, head):
            """Score matmuls + exp (band tiles first, suffix-trimmed)."""
            kt_h = kts[head]
            q_sl = qts[head][:, qr * 512:(qr + 1) * 512]
            pts = {}
            band = list(range(4 * qr, 4 * qr + 4))
            old = list(range(0, 4 * qr))
            for kt in band + old:
                i = kt - 4 * qr
                c0 = i * 128 if i >= 0 else 0
                ps_t = ps.tile([128, 512], F32, tag="b",
                               name=f"st{qr}_{head}_{kt}", bufs=2)
                nc.tensor.matmul(
                    ps_t[:, c0:], kt_h[:, kt * 128:(kt + 1) * 128],
                    q_sl[:, c0:])
                pool = aptA if kt < 8 else aptB
                pT = pool.tile([128, 512], BF16, tag="pT",
                               name=f"pT{qr}_{head}_{kt}")
                if i >= 0:
                    # diagonal block: exp then 0/1 triangle multiply
                    nc.scalar.activation(
                        pT[:, c0:c0 + 128], ps_t[:, c0:c0 + 128],
                        mybir.ActivationFunctionType.Exp, scale=SCALE)
                    nc.vector.tensor_mul(
                        pT[:, c0:c0 + 128], pT[:, c0:c0 + 128], tri_sb[:])
                    if i < 3:
                        nc.scalar.activation(
                            pT[:, c0 + 128:], ps_t[:, c0 + 128:],
                            mybir.ActivationFunctionType.Exp, scale=SCALE)
                else:
                    nc.scalar.activation(
                        pT[:], ps_t[:],
                        mybir.ActivationFunctionType.Exp, scale=SCALE)
                pts[kt] = pT
            return pts

        def emit_pv(qr, head, pts):
            """PV accumulation + exp-sum partials. Returns the pending
            softmax-finish state (flushed later, pipelined)."""
            nkt = 4 * qr + 4
            band = list(range(4 * qr, 4 * qr + 4))
            old = list(range(0, 4 * qr))
            # exp-sum chains over the full-width old tiles (vector + gpsimd)
            accs = []
            parts = [old[0::2], old[1::2]] if len(old) > 2 else [old]
            engs = [nc.vector, nc.gpsimd]
            for pi, part in enumerate(parts):
                if not part:
                    continue
                eng = engs[pi]
                acc = awk.tile([128, 512], BF16, tag=f"acc{pi}",
                               name=f"acc{pi}_{qr}_{head}", bufs=1)
                if len(part) >= 2:
                    eng.tensor_add(acc[:], pts[part[0]][:], pts[part[1]][:])
                else:
                    eng.tensor_copy(acc[:], pts[part[0]][:])
                for kt in part[2:]:
                    eng.tensor_add(acc[:], acc[:], pts[kt][:])
                accs.append(acc)
            # PV accumulation (old tiles full width, band tiles suffix)
            ps_pv = ps.tile([128, 512], F32, tag="c",
                            name=f"pv{qr}_{head}", bufs=2)
            for kt in range(nkt):
                i = kt - 4 * qr
                c0 = i * 128 if i >= 0 else 0
                nc.tensor.matmul(
                    ps_pv[:, c0:], vall[:, kt, head * 128:(head + 1) * 128],
                    pts[kt][:, c0:],
                    start=(kt == 0), stop=(kt == nkt - 1))
            # denominator: band tiles summed on the PE (suffix ones-matmuls)
            ps_rsb = ps.tile([128, 512], F32, tag="c",
                             name=f"rsb{qr}_{head}", bufs=2)
            for bi, kt in enumerate(band):
                c0 = (kt - 4 * qr) * 128
                nc.tensor.matmul(
                    ps_rsb[:, c0:], ones_sb[:], pts[kt][:, c0:],
                    start=(bi == 0), stop=(bi == 3 and not accs))
            return (qr, head, ps_pv, ps_rsb, accs)

        def emit_finish(pend):
            """Rowsum of the acc chains + reciprocal + divide + agi store."""
            qr, head, ps_pv, ps_rsb, accs = pend
            for ai, acc in enumerate(accs):
                nc.tensor.matmul(ps_rsb[:], ones_sb[:], acc[:],
                                 start=False, stop=(ai == len(accs) - 1))
            rec_bc = awk.tile([128, 512], F32, tag="recb",
                              name=f"rec{qr}_{head}", bufs=1)
            nc.vector.reciprocal(rec_bc[:], ps_rsb[:])
            at_sb = awk.tile([128, 512], BF16, tag="at",
                             name=f"at{qr}_{head}")
            nc.vector.tensor_mul(at_sb[:], ps_pv[:], rec_bc[:])
            nc.gpsimd.dma_start(
                agi[qr][head * 128:(head + 1) * 128, :], at_sb[:])

        def emit_ag(qr):
            nc.gpsimd.collective_compute(
                "AllGather",
                mybir.AluOpType.bypass,
                replica_groups=[list(range(N_CORES))],
                ins=[agi[qr][:].opt()],
                outs=[ago[qr][:].opt()],
            )

        # ---------- WO units ----------
        def make_wo_units(r):
            """32 chunk-closures of 4 MMs each (hh, qtl, dhalf) + an output
            unit per qtl pair (PSUM->SBUF copy + DRAM store)."""
            wo_sb = cpools["wo_sb"]
            ps_os = [ps.tile([128, 512], F32, tag=f"a{qtl}",
                             name=f"wops{r}_{qtl}", bufs=1)
                     for qtl in range(4)]
            atqfs = {}

            def load(hh):
                if hh > 3 or hh in atqfs:
                    return
                atqf = cpools["woa"].tile(
                    [128, 8, 512], BF16, tag="atqf", name=f"atqf{r}_{hh}")
                nc.sync.dma_start(
                    atqf[:],
                    ago[r][hh * 1024:(hh + 1) * 1024, :]
                    .rearrange("(dt p) q -> p dt q", p=128),
                )
                atqfs[hh] = atqf

            def outq(qtl):
                qt = r * 4 + qtl
                o_sb = cpools["woo"].tile([128, 512], F32, tag="osb",
                                          name=f"osb{qt}")
                nc.scalar.copy(o_sb[:], ps_os[qtl][:])
                nc.sync.dma_start(
                    out_d[qt * 128:(qt + 1) * 128, :], o_sb[:])

            units = []
            for hh in range(4):
                for qtl in range(4):
                    for dh in range(2):
                        def chunk(hh=hh, qtl=qtl, dh=dh):
                            if qtl == 0 and dh == 0:
                                load(hh)
                            if qtl == 2 and dh == 0:
                                load(hh + 1)
                            atqf = atqfs[hh]
                            for dt in range(dh * 4, dh * 4 + 4):
                                gdt = hh * 8 + dt
                                nc.tensor.matmul(
                                    ps_os[qtl][:],
                                    atqf[:, dt, qtl * 128:(qtl + 1) * 128],
                                    wo_sb[:, gdt, :],
                                    start=(gdt == 0), stop=(gdt == KT - 1))
                            if hh == 3 and dh == 1:
                                outq(qtl)
                        units.append(chunk)
            return units

        # ---------- emission schedule ----------
        def xt_direct(vq):
            def src(kt, tt):
                base = vq * 512 + tt * 128
                return xt_sb[:, kt, base:base + 128]
            return src

        fillers = []

        def fill(n):
            for _ in range(n):
                if fillers:
                    fillers.pop(0)()

        def drain():
            while fillers:
                fillers.pop(0)()

        pending = [None]

        def attn_head(qr, head, nfill=3, npost=3):
            pts = emit_scores(qr, head)
            if pending[0] is not None:
                emit_finish(pending[0])
            fill(nfill)
            pending[0] = emit_pv(qr, head, pts)
            fill(npost)

        def flush_and_ag(qr):
            emit_finish(pending[0])
            pending[0] = None
            emit_ag(qr)

        # vq0 fully upfront (PV of qr=0 needs tokens 0..511)
        for u in make_vq_units(0, xt_direct(0)):
            u()

        fillers = make_vq_units(1, xt_direct(1))
        for head in range(4):
            attn_head(0, head, nfill=6, npost=2)
        flush_and_ag(0)
        drain()

        fillers = make_vq_units(2, xt_direct(2))
        for head in range(4):
            attn_head(1, head)
        flush_and_ag(1)
        drain()

        fillers = make_vq_units(3, xt_direct(3))
        for head in range(4):
            attn_head(2, head)
        flush_and_ag(2)
        drain()

        # x^T no longer needed: release its 128KB (left side)
        xtp_cm.__exit__(None, None, None)

        # phase-C pools in the freed left space
        wop = es.enter_context(tc.tile_pool(name="wop", bufs=1, side="left"))
        woa = es.enter_context(tc.tile_pool(name="woa", bufs=2, side="left"))
        woo = es.enter_context(tc.tile_pool(name="woo", bufs=2, side="left"))
        wo_sb = wop.tile([128, KT, DSH], BF16, tag="wo")
        cpools["wo_sb"] = wo_sb
        cpools["woa"] = woa
        cpools["woo"] = woo
        for ch in range(8):
            nc.sync.dma_start(
                wo_sb[:, ch * 4:(ch + 1) * 4, :],
                wo_d[:, ch * 4:(ch + 1) * 4, :],
            )

        # attn(3) with wo(0) chunks as fillers; ag(0) completed long ago
        fillers = make_wo_units(0)
        for head in range(4):
            attn_head(3, head)
        flush_and_ag(3)
        drain()           # rest of wo(0) — runs while ag(3) is on the wire
        for u in make_wo_units(1):
            u()
        for u in make_wo_units(2):
            u()
        for u in make_wo_units(3):
            u()

    nc.compile()
    return nc


def _get_program(mode):
    if mode not in _PROGRAMS:
        if mode == "causal":
            _PROGRAMS[mode] = _build_causal()
        else:
            from kernel_baseline import _build_program as _legacy
            _PROGRAMS[mode] = _legacy(mode)
    return _PROGRAMS[mode]


def _prep_inputs(x, wq, wk, wv, wo, freqs_real, freqs_imag, mask):
    """Host-side shard/layout prep. Returns (mode, in_maps)."""
    x = np.asarray(x, dtype=np.float32)
    wq = np.asarray(wq, dtype=np.float32)
    wk = np.asarray(wk, dtype=np.float32)
    wv = np.asarray(wv, dtype=np.float32)
    wo = np.asarray(wo, dtype=np.float32)
    fr = np.asarray(freqs_real, dtype=np.float32)
    fi = np.asarray(freqs_imag, dtype=np.float32)
    m = np.asarray(mask, dtype=np.float32).reshape(S, S)

    causal_ref = np.triu(np.full((S, S), np.float32(-1e9), dtype=np.float32), k=1)
    if np.array_equal(m, causal_ref):
        mode = "causal"
    elif not m.any():
        mode = "nomask"
    else:
        mode = "general"

    if mode != "causal":
        from kernel_baseline import _prep_inputs as _legacy_prep
        return _legacy_prep(x, wq, wk, wv, wo, freqs_real, freqs_imag, mask)

    xT = np.ascontiguousarray(x.reshape(S, D).T)  # [D, S]
    xT_bf = xT.astype(ml_dtypes.bfloat16)

    # evens-first permutation of each head's 128 dims (for RoPE pair layout)
    idx = np.concatenate([np.arange(0, HD, 2), np.arange(1, HD, 2)])
    cols = np.concatenate([h * HD + idx for h in range(32)])
    wq_p = wq[:, cols]
    wk_p = wk[:, cols]

    fr128 = np.ascontiguousarray(np.concatenate([fr.T, fr.T], axis=0))   # [128, S]
    fis128 = np.ascontiguousarray(np.concatenate([-fi.T, fi.T], axis=0))

    perm = np.zeros((128, 128), dtype=np.float32)
    perm[np.arange(128), (np.arange(128) + 64) % 128] = 1.0

    onesmat = np.ones((128, 128), dtype=np.float32)
    # multiplicative causal tile mask in [k, q] layout: 1 iff k <= q
    tri01 = (np.arange(128)[:, None] <= np.arange(128)[None, :]).astype(np.float32)

    in_maps = []
    for c in range(N_CORES):
        sl = slice(c * DSH, (c + 1) * DSH)

        def _wtile(a):
            # [D, C] -> [128p, KT, C] matching the SBUF tile layout
            return np.ascontiguousarray(
                a.reshape(KT, 128, a.shape[1]).transpose(1, 0, 2)
            ).astype(ml_dtypes.bfloat16)

        def _whead(a):
            # [D, 512] -> [NH_LOC, 128p, KT, HD]
            return np.ascontiguousarray(np.stack([
                _wtile(a[:, h * HD:(h + 1) * HD]) for h in range(NH_LOC)
            ]))

        im = {
            "xT": xT_bf,
            "wq": _whead(wq_p[:, sl]),
            "wk": _whead(wk_p[:, sl]),
            "wv": _wtile(wv[:, sl]),
            "wo": _wtile(wo[:, sl]),
            "perm": perm,
            "fr128": fr128.astype(ml_dtypes.bfloat16),
            "fis128": fis128.astype(ml_dtypes.bfloat16),
            "onesmat": onesmat.astype(ml_dtypes.bfloat16),
            "tri01": tri01.astype(ml_dtypes.bfloat16),
        }
        in_maps.append(im)
    return mode, in_maps


def kernel(x, wq, wk, wv, wo, cache_k, cache_v, freqs_real, freqs_imag,
           mask, start_pos, **_unused):
    assert int(start_pos) == 0, "kernel hardcodes start_pos=0"
    mode, in_maps = _prep_inputs(x, wq, wk, wv, wo, freqs_real, freqs_imag, mask)
    nc = _get_program(mode)
    res = run_bass_kernel_spmd(nc, in_maps, core_ids=list(range(N_CORES)))
    out = np.concatenate([res.results[c]["out"] for c in range(N_CORES)], axis=1)
    return out.reshape(1, S, D).astype(np.float32)
